# revision 1
# baseline (speedup 1.0000x reference)
"""Trainium2 Bass kernel for the DyadBlock problem.

Math (reference):
    xb   = x.reshape(DY, DI, B)
    incl = cumsum(xb, axis=0)             # inclusive prefix over dyads
    total= incl[-1]
    out[d] = w_lower[d] @ incl[d] + w_upper[d] @ (total - incl[d]) + bias

Rewrite (wd = w_lower - w_upper):
    out[d] = wd[d] @ incl[d] + w_upper[d] @ total + bias

Device strategy (pure data parallel over batch, 8 cores; each core owns a
1024-column batch slice processed as 2 x 512-column chunks):
  - dyads are processed 4 at a time as one 128-partition tile (64 tiles).
  - the cumsum runs on the TensorEngine:
       m1: incl_tile  = LT4.T @ x_tile             (block-triangular of I32)
       m2: incl_tile += O4I.T @ prev_incl[96:128]  (carry broadcast, K=32
           row-strip 3)
    The 64 tiles are split into 4 independent carry chains of 16 tiles
    (quarters); the missing inter-quarter carry is folded into m4 below.
  - per-tile output:
       m4: pout  = WQ[t].T @ Estack      K=128; Estack rows 32p hold the
           quarter-end prefixes E_p, and WQ[t] rows are
           w_upper[d].T + (p < quarter(t)) * wd[d].T -- this one matmul
           applies BOTH the shared-total term (sum of all E_p) and the
           inter-quarter carry fixup (sum of E_p for p < quarter).
       m3: pout += WDbd[t].T @ incl_local[t]   (block-diagonal, K=128)
       out = pout + bias (ScalarE/VectorE alternating), DMA out.
  - fp16 operands by default: 1 cycle/row on the PE (4x faster than fp32)
    with an 11-bit mantissa; all value ranges here fit fp16 comfortably.
  - x is loaded as full-width [128, 1024] contiguous DMAs (issued on
    gpsimd); outputs stored as full-width DMAs (issued on sync).
"""

import os

import numpy as np

import concourse.bacc as bacc
import concourse.mybir as mybir
import concourse.tile as tile
from concourse import bass_utils
from concourse.tile_rust import add_dep_helper

DY, DO, DI = 256, 32, 32
B = 8192
NCORES = 8
BC = B // NCORES  # batch columns per core
NQ = 4            # carry chains (quarters) per chunk

_cache = {}
last_results = None


def _cfg():
    mm = os.environ.get("DYAD_MM_DT", "fp16")
    mm_dt = {
        "f32": mybir.dt.float32,
        "f32r": mybir.dt.float32r,
        "bf16": mybir.dt.bfloat16,
        "fp16": mybir.dt.float16,
    }[mm]
    out_s = os.environ.get("DYAD_OUT_DT", "same")
    out_dt = {"f32": mybir.dt.float32, "same": mm_dt}[out_s]
    return mm_dt, out_dt


def build(mm_dt, out_dt, bc=BC, n=512, nt=DY // 4, x_bufs=14, o_bufs=6,
          pi_bufs=4, po_bufs=4):
    """Build + compile the per-core Bass program."""
    f32 = mybir.dt.float32
    nchunk = bc // n
    ql = nt // NQ  # tiles per carry chain
    nc = bacc.Bacc("TRN2", target_bir_lowering=False, debug=False,
                   num_devices=NCORES)
    x_d = nc.dram_tensor("x", [nt * 128, bc], mm_dt, kind="ExternalInput").ap()
    # wd4 ships compact ([4][32, nt*32] diagonal blocks); it is expanded
    # into the zeroed block-diagonal SBUF tile by 4 strided-AP DMAs.
    wd4_d = nc.dram_tensor("wd4", [4, 32, nt * 32], mm_dt,
                           kind="ExternalInput").ap()
    wq_d = nc.dram_tensor("wq", [128, nt * 128], mm_dt,
                          kind="ExternalInput").ap()
    lt4_d = nc.dram_tensor("lt4", [128, 128], mm_dt, kind="ExternalInput").ap()
    m2full = os.environ.get("DYAD_M2FULL", "1") == "1"
    o4i_d = nc.dram_tensor("o4i", [128 if m2full else 32, 128], mm_dt,
                           kind="ExternalInput").ap()
    bias_d = nc.dram_tensor("biast", [128, nt], f32, kind="ExternalInput").ap()
    out_d = nc.dram_tensor("out", [nt * 128, bc], out_dt,
                           kind="ExternalOutput").ap()

    with tile.TileContext(nc) as tc:
        with tc.tile_pool(name="wpool", bufs=1) as wp, \
             tc.tile_pool(name="xpool", bufs=x_bufs) as xp, \
             tc.tile_pool(name="opool", bufs=o_bufs) as op, \
             tc.tile_pool(name="ipool", bufs=nchunk * nt) as ip, \
             tc.tile_pool(name="epool", bufs=nchunk) as ep, \
             tc.tile_pool(name="pincl_pool", bufs=pi_bufs, space="PSUM") as pip, \
             tc.tile_pool(name="pout_pool", bufs=po_bufs, space="PSUM") as pop:
            wd4 = wp.tile([128, nt * 128], mm_dt)
            wq = wp.tile([128, nt * 128], mm_dt)
            lt4 = wp.tile([128, 128], mm_dt)
            o4i = wp.tile([128, 128], mm_dt)
            biast = wp.tile([128, nt], f32)
            # weights stream first, serially, before phase A kicks off:
            # concurrent bulk streams (even on different queues) repeatedly
            # measured ~2x slower overall -- likely DRAM page thrash -- so
            # keep one stream at a time.
            nc.vector.memset(wd4[:], 0.0)
            for a in range(4):
                # SBUF dst: partitions 32a..32a+32, cols t*128 + 32a + i
                nc.sync.dma_start(
                    out=wd4[32 * a:32 * (a + 1), :].rearrange(
                        "p (t i) -> p t i", i=128)[:, :, 32 * a:32 * (a + 1)],
                    in_=wd4_d[a].rearrange("p (t i) -> p t i", i=32))
            nc.sync.dma_start(out=lt4[:], in_=lt4_d)
            if m2full:
                nc.sync.dma_start(out=o4i[:], in_=o4i_d)
            else:
                nc.sync.dma_start(out=o4i[96:128, :], in_=o4i_d)
            nc.sync.dma_start(out=biast[:], in_=bias_d)
            # wq heads the gpsimd queue: it serializes IN FRONT of the x
            # stream (no concurrent-stream thrash) while phase A's first
            # matmuls only wait for the small sync-queue constants.
            nc.gpsimd.dma_start(out=wq[:], in_=wq_d)

            # ---- phase A: 2*NQ interleaved carry chains ----
            incl_all = [[None] * nt for _ in range(nchunk)]
            prev = [[None] * nchunk for _ in range(NQ)]
            copy_flip = 0
            # x DMA issue order: chain-interleaved (consumption order,
            # default) or burst-sequential runs of 4 consecutive tiles for
            # DRAM page locality (DYAD_XORDER=burst).
            if os.environ.get("DYAD_XORDER", "il") == "burst" and ql >= 4:
                issue_order = [q * ql for q in range(NQ)]
                for q in range(NQ):
                    issue_order += [q * ql + s for s in range(1, 4)]
                for base in range(4, ql, 4):
                    for q in range(NQ):
                        issue_order += [q * ql + s
                                        for s in range(base, base + 4)]
            else:
                issue_order = [q * ql + s for s in range(ql)
                               for q in range(NQ)]
            xts = {}
            for tt in issue_order:
                xt = xp.tile([128, bc], mm_dt, tag="x", name=f"x_{tt}")
                nc.gpsimd.dma_start(
                    out=xt[:], in_=x_d[128 * tt:128 * (tt + 1), :])
                xts[tt] = xt
            for step in range(ql):
                for q in range(NQ):
                    tt = q * ql + step
                    xt = xts[tt]
                    # group same-weight matmuls (m1,m1 then m2,m2) so the PE
                    # isn't reloading lt4/o4i on every single matmul
                    pincls, m1s = [], []
                    for c in range(nchunk):
                        pincl = pip.tile([128, n], f32, tag="pincl",
                                         name=f"pincl_{c}_{tt}")
                        m1 = nc.tensor.matmul(
                            pincl[:], lt4[:], xt[:, c * n:(c + 1) * n],
                            start=True, stop=(step == 0), tile_position=(0, 0))
                        pincls.append(pincl)
                        m1s.append(m1)
                    for c in range(nchunk):
                        if step > 0:
                            if m2full:
                                # uniform full-array matmul: rows 0:96 of the
                                # padded weights are zero, so only the carry
                                # rows of the previous incl tile contribute
                                m2 = nc.tensor.matmul(
                                    pincls[c][:], o4i[:], prev[q][c][:],
                                    start=False, stop=True,
                                    tile_position=(0, 0))
                            else:
                                m2 = nc.tensor.matmul(
                                    pincls[c][:], o4i[96:128, :],
                                    prev[q][c][96:128, :],
                                    start=False, stop=True,
                                    tile_position=(96, 0))
                            add_dep_helper(m2.ins, m1s[c].ins, sync=False,
                                           reason="psum group order m1->m2")
                    for c in range(nchunk):
                        incl = ip.tile([128, n], mm_dt, tag="incl",
                                       name=f"incl_{c}_{tt}")
                        if copy_flip == 0:
                            nc.vector.tensor_copy(out=incl[:], in_=pincls[c][:])
                        else:
                            nc.scalar.copy(out=incl[:], in_=pincls[c][:])
                        copy_flip ^= 1
                        prev[q][c] = incl
                        incl_all[c][tt] = incl

            # ---- quarter-end prefix stacks (SBUF->SBUF DMA: cross-partition) ----
            estack = []
            for c in range(nchunk):
                est = ep.tile([128, n], mm_dt, tag="estack", name=f"estack_{c}")
                for q in range(NQ):
                    nc.sync.dma_start(
                        out=est[32 * q:32 * (q + 1), :],
                        in_=incl_all[c][q * ql + ql - 1][96:128, :])
                estack.append(est)

            # ---- phase B: outputs ----
            add_flip = 0
            for tt in range(nt):
                outt = op.tile([128, bc], out_dt, tag="out", name=f"out_{tt}")
                for c in range(nchunk):
                    pout = pop.tile([128, n], f32, tag="pout",
                                    name=f"pout_{c}_{tt}")
                    m4 = nc.tensor.matmul(
                        pout[:], wq[:, 128 * tt:128 * (tt + 1)], estack[c][:],
                        start=True, stop=False, tile_position=(0, 0))
                    m3 = nc.tensor.matmul(
                        pout[:], wd4[:, 128 * tt:128 * (tt + 1)],
                        incl_all[c][tt][:],
                        start=False, stop=True, tile_position=(0, 0))
                    add_dep_helper(m3.ins, m4.ins, sync=False,
                                   reason="psum group order m4->m3")
                    dst = outt[:, c * n:(c + 1) * n]
                    if add_flip == 0:
                        nc.scalar.add(out=dst, in_=pout[:],
                                      add=biast[:, tt:tt + 1])
                    else:
                        nc.vector.tensor_scalar_add(out=dst, in0=pout[:],
                                                    scalar1=biast[:, tt:tt + 1])
                    add_flip ^= 1
                nc.sync.dma_start(
                    out=out_d[128 * tt:128 * (tt + 1), :], in_=outt[:])
    nc.compile()
    return nc


def host_weights(w_upper, w_lower, bias, np_io, nt=DY // 4):
    """Host-side constant/weight layouts (lhsT conventions, see build())."""
    w_upper = np.asarray(w_upper, dtype=np.float32)
    w_lower = np.asarray(w_lower, dtype=np.float32)
    bias = np.asarray(bias, dtype=np.float32)
    ql = nt // NQ
    wd = w_lower - w_upper
    wdT = wd.transpose(0, 2, 1)    # [d, j, i] = wd[d][i, j]
    wuT = w_upper.transpose(0, 2, 1)
    wdr = wdT.reshape(nt, 4, 32, 32)   # [t, a, j, i]
    wur = wuT.reshape(nt, 4, 32, 32)

    # compact diagonal blocks: WD4[a][j, t*32+i] = wd[4t+a][i,j]; the
    # kernel expands them into the zeroed block-diagonal SBUF tile.
    arr = np.zeros((4, 32, nt, 32), np.float32)
    for a in range(4):
        arr[a] = wdr[:, a].transpose(1, 0, 2)
    WD4 = np.ascontiguousarray(arr.reshape(4, 32, nt * 32))

    # WQ: rows 32p+j, cols t*128+a*32+i =
    #     wu[4t+a][i,j] + (p < t // ql) * wd[4t+a][i,j]
    W = np.zeros((4, 32, nt, 4, 32), np.float32)
    quarter = (np.arange(nt) // ql)
    for p in range(NQ):
        blk = wur + ((quarter > p).astype(np.float32))[:, None, None, None] * wdr
        W[p] = blk.transpose(2, 0, 1, 3)   # [j, t, a, i]
    WQ = np.ascontiguousarray(W.reshape(128, nt * 128))

    LT4 = np.kron(np.triu(np.ones((4, 4), np.float32)),
                  np.eye(32, dtype=np.float32))
    O4I = np.tile(np.eye(32, dtype=np.float32), (1, 4))
    if os.environ.get("DYAD_M2FULL", "1") == "1":
        O4I = np.vstack([np.zeros((96, 128), np.float32), O4I])
    BIAST = np.ascontiguousarray(
        bias.reshape(nt, 4, 32).transpose(1, 2, 0).reshape(128, nt))
    return {
        "wd4": WD4.astype(np_io, copy=False),
        "wq": WQ.astype(np_io, copy=False),
        "lt4": LT4.astype(np_io, copy=False),
        "o4i": O4I.astype(np_io, copy=False),
        "biast": BIAST,
    }


def _run_profiled(nc, in_maps):
    """Mirror of bass_utils' axon trace branch; the antenv.axon_hooks
    module is absent in this image, so drive the ctypes NTFF hook from
    trn_agent_boot directly and post-process with bass_utils helpers."""
    import glob
    import tempfile

    import gauge.profiler
    from concourse import bass2jax
    from concourse._compat import FishPath
    from trn_agent_boot.trn_boot import _ntff_profile_via_ctypes

    hook = _ntff_profile_via_ctypes("/opt/axon/libaxon_pjrt.so")
    if hook is None:
        raise RuntimeError("no NTFF profile symbols in libaxon_pjrt.so")
    neff_dir = tempfile.mkdtemp(prefix="dyad_prof_")
    with hook(neff_dir, [0]):
        results = bass2jax.run_bass_via_pjrt(nc, in_maps, n_cores=NCORES)
    ntffs = glob.glob(os.path.join(neff_dir, "*_body*.ntff"))
    if not ntffs:
        raise RuntimeError(f"no NTFFs in {neff_dir}")
    profile = gauge.profiler.Profile(
        profile_path=FishPath(neff_dir),
        kernel_dev_mode=True,
        profile_on_exit=False,
        bass_kernel=nc.m,
        offline_processing=True,
        fname="*_body*",
        metadata={},
    )
    return bass_utils._process_ntff_profile(
        profile, neff_dir, nc, list(range(NCORES)), [0], False, {},
        trace_events=False,
    ).as_bass_kernel_results(results)


def kernel(x, w_upper, w_lower, bias):
    global last_results
    mm_dt, out_dt = _cfg()
    key = (mm_dt, out_dt)
    if key not in _cache:
        _cache[key] = build(mm_dt, out_dt)
    nc = _cache[key]

    np_io = mybir.dt.np(mm_dt)
    x = np.asarray(x, dtype=np.float32)
    w = host_weights(w_upper, w_lower, bias, np_io)
    in_maps = []
    for cidx in range(NCORES):
        xs = np.ascontiguousarray(x[:, cidx * BC:(cidx + 1) * BC]).astype(
            np_io, copy=False)
        in_maps.append({"x": xs, **w})

    if os.environ.get("DYAD_TRACE", "0") == "1":
        try:
            res = _run_profiled(nc, in_maps)
        except Exception as e:  # profiling is best-effort
            print("profiled run failed (%s); falling back" % e)
            res = bass_utils.run_bass_kernel_spmd(
                nc, in_maps, core_ids=list(range(NCORES)), trace=False)
    else:
        res = bass_utils.run_bass_kernel_spmd(
            nc, in_maps, core_ids=list(range(NCORES)), trace=False)
    last_results = res
    out = np.concatenate([res.results[c]["out"] for c in range(NCORES)], axis=1)
    return np.ascontiguousarray(out, dtype=np.float32)

